# revision 33
# baseline (speedup 1.0000x reference)
"""Trainium2 Bass kernel for nn_Decoder_43696997269791.

Math (validated against the reference in fp64, rel err 2e-7):
  scores  = (enc @ enc^T) / TEMP                   per sample, [L, L], symmetric
  attn    = tanh(scores)          (mask is all-ones per the spec -> identity)
  seq1    = mean_l(attn @ enc)    = (rowsum(attn)/L) @ enc   (attn symmetric)
  conv branch: both convs are linear -> seq2[d] = sum_j u_j[d+j-1] + const,
      u_j = W3u[:, j]^T @ enc  with  W3u[l, j] = sum_i conv_w[i,j]*w3[l+1-i]
  out = tanh(user + seq1/2 + 2*seq2)

Device mapping (8 NeuronCores, data-parallel over batch, 8 samples/core):
  - tanh(scores) is symmetric: only upper-triangle block strips are computed.
    Upper row sums come from the ScalarE activation's accum_out or a VectorE
    reduce; the missing lower part of each row sum equals column sums of the
    strips, accumulated with ones-vector matmuls into [1, m] PSUM rows.
  - the colsum row is transposed back to partition layout PER 128-COLUMN
    BLOCK (32x32 VectorE block transpose + tiny gathers) as soon as that
    block's colsum is final, so each block's fused-matmul stationary column
    is ready one strip after the block's last contribution - the per-sample
    tail matmuls interleave into their own sample's strip stream and the
    end-of-kernel serial chain is short.
  - seq1 + both conv terms come from one fused matmul whose stationary puts
    its three weight columns at 0/32/64; the shifted mix runs column-halved
    on ScalarE+VectorE; the user embedding rides in encN row 704 so the
    matmul accumulates it for free.
  - all DRAM operands are host-pre-shuffled so every DMA is a single
    [128 x contiguous-bytes] transfer (minimal descriptor-generation time)
  - dummy warm-up matmuls cover the initial DMA window so HAM un-throttles
    before real matmuls arrive
"""

import sys

import numpy as np
import ml_dtypes

sys.path.insert(0, "/opt/trn_rl_repo")

B, L, D = 64, 700, 512
LP = 704            # L padded to DMA/partition-friendly multiple
LW = 768            # W3u rows (and the transposed colsum row) padded to 6*128
NCORES = 8
BPC = B // NCORES   # samples per core
TEMP = float(np.sqrt(512.0))
NLB = 6             # number of 128-row l-blocks in LP (last block is 64)
LBS = [min(128, LP - 128 * i) for i in range(NLB)]
N_WARMUP_MM = 22
RSCALE = 1.0 / (2.0 * L)
SW = 65             # fused-matmul stationary width: real columns at 0/32/64

_PROG = None


def _build_program():
    import concourse.mybir as mybir
    import concourse.tile as tile
    from concourse import bacc

    f32 = mybir.dt.float32
    bf16 = mybir.dt.bfloat16
    Tanh = mybir.ActivationFunctionType.Tanh
    ADD = mybir.AluOpType.add
    MULT = mybir.AluOpType.mult

    nc = bacc.Bacc(None, target_bir_lowering=False)
    encN = nc.declare_dram_parameter("encN", [BPC, 128, NLB, D], bf16,
                                     isOutput=False)
    encT = nc.declare_dram_parameter("encT", [BPC, 128, 4, LP], bf16,
                                     isOutput=False)
    w3u = nc.declare_dram_parameter("w3u", [128, NLB, 3], bf16, isOutput=False)
    w3ct = nc.declare_dram_parameter("w3ct", [1, LW], f32, isOutput=False)
    out = nc.declare_dram_parameter("out", [1, BPC * D], f32, isOutput=True)

    with tile.TileContext(nc) as tc:
        with (
            tc.tile_pool(name="const", bufs=1) as constp,
            tc.tile_pool(name="enc", bufs=3) as encp,
            tc.tile_pool(name="work", bufs=2) as workp,
            tc.tile_pool(name="ps_s", bufs=4, space="PSUM") as ps_s,
            tc.tile_pool(name="ps_u", bufs=2, space="PSUM") as ps_u,
            tc.tile_pool(name="ps_sl", bufs=1, space="PSUM") as ps_sl,
        ):
            # ---- sample 0's enc tiles first: the ScalarE DGE queue is past
            # its preamble earliest, so these transfers start several us sooner
            encTt0 = encp.tile([128, 4, LP], bf16, tag="encTt", name="encTt0")
            nc.scalar.dma_start(out=encTt0[:, :, :], in_=encT[0])
            encNt0 = encp.tile([128, NLB, D], bf16, tag="encNt", name="encNt0")
            nc.scalar.dma_start(out=encNt0[:, :, :], in_=encN[0])

            # ---- PE warm-up: keep the array busy through the initial DMA
            # window so HAM un-throttles before real matmuls arrive
            wsrc = constp.tile([128, 512], bf16, tag="wsrc", name="wsrc")
            nc.gpsimd.memset(wsrc[:, :], 0.0)
            wps = ps_u.tile([SW, 512], f32, tag="psu", name="wps")
            for _ in range(N_WARMUP_MM):
                nc.tensor.matmul(wps[0:1, 0:256], wsrc[:, 0:1], wsrc[:, 0:256],
                                 start=True, stop=True)

            w3u_sb = constp.tile([128, NLB, 3], bf16, tag="w3u_sb", name="w3u_sb")
            nc.sync.dma_start(out=w3u_sb[:, :, :], in_=w3u[:, :, :])
            out_sb = constp.tile([1, BPC * D], f32, tag="out_sb", name="out_sb")
            # explicit zero bias for Tanh activations: a float bias would pull
            # in a const-AP DMA and push the instruction over the sync-wait cap
            zbias = constp.tile([128, 1], f32, tag="zbias", name="zbias")
            nc.vector.memset(zbias[:, :], 0.0)
            ones_sb = constp.tile([128, 1], bf16, tag="ones_sb", name="ones_sb")
            nc.vector.memset(ones_sb[:, :], 1.0)
            # bounce rows 1-31 stay zero forever; row 0 carries the scaled
            # lower colsums plus the center-tap weight 2*W3u[:,1] in free
            # layout (cols 128:704 rewritten per sample, the rest constant)
            w3ct_sb = constp.tile([1, LW], f32, tag="w3ct_sb", name="w3ct_sb")
            nc.sync.dma_start(out=w3ct_sb[0:1, :], in_=w3ct[0:1, :])
            bounce = constp.tile([32, LW], f32, tag="bounce", name="bounce")
            nc.gpsimd.memset(bounce[:, :], 0.0)
            nc.vector.tensor_copy(out=bounce[0:1, 0:128],
                                  in_=w3ct_sb[0:1, 0:128])
            nc.vector.tensor_copy(out=bounce[0:1, 704:768],
                                  in_=w3ct_sb[0:1, 704:768])
            outT = constp.tile([32, LW], f32, tag="outT", name="outT")
            nc.gpsimd.memset(outT[:, :], 0.0)

            # ping-pong tiles (distance-2 reuse; avoids per-sample memsets)
            r6cs, stats = [], []
            for pp in range(2):
                r6c = constp.tile([128, NLB, 2], f32, tag=f"r6c{pp}",
                                  name=f"r6c{pp}")
                nc.vector.memset(r6c[:, :, :], 0.0)
                r6cs.append(r6c)
                # stationary: col0 = r/(2L) + 2*W3u[:,1] (rewritten per sample,
                # block by block), col32 = 2*W3u[:,0], col64 = 2*W3u[:,2]
                stat = constp.tile([128, NLB, SW], bf16, tag=f"stat{pp}",
                                   name=f"stat{pp}")
                nc.gpsimd.memset(stat[:, :, :], 0.0)
                nc.vector.tensor_copy(out=stat[:, :, 32:33],
                                      in_=w3u_sb[:, :, 0:1])
                nc.vector.tensor_copy(out=stat[:, :, 64:65],
                                      in_=w3u_sb[:, :, 2:3])
                stats.append(stat)

            # ---- previous-sample mix/out work, injected into this sample's
            # strip stream (no PE ops except the two tail matmuls at slot 0)
            def prev_tail45(pv):
                for lb in (4, 5):
                    K = LBS[lb] + (1 if lb == NLB - 1 else 0)
                    nc.tensor.matmul(
                        pv["psu"][:, :],
                        pv["stat"][0:K, lb, :],
                        pv["encNt"][0:K, lb, :],
                        start=False, stop=(lb == NLB - 1),
                    )

            def prev_mix(pv, half):
                # t1[d] = psu0[d] + psu32[d-1] + psu64[d+1] on this half
                psu = pv["psu"]
                if half == 0:
                    t1 = workp.tile([1, 512], f32, tag="t1", name="t1")
                    pv["t1"] = t1
                    nc.scalar.copy(out=t1[0:1, 0:256], in_=psu[0:1, 0:256])
                    nc.vector.tensor_tensor(
                        out=t1[0:1, 1:256], in0=t1[0:1, 1:256],
                        in1=psu[32:33, 0:255], op=ADD,
                    )
                    nc.vector.tensor_tensor(
                        out=t1[0:1, 0:256], in0=t1[0:1, 0:256],
                        in1=psu[64:65, 1:257], op=ADD,
                    )
                else:
                    t1 = pv["t1"]
                    nc.scalar.copy(out=t1[0:1, 256:512], in_=psu[0:1, 256:512])
                    nc.vector.tensor_tensor(
                        out=t1[0:1, 256:512], in0=t1[0:1, 256:512],
                        in1=psu[32:33, 255:511], op=ADD,
                    )
                    nc.vector.tensor_tensor(
                        out=t1[0:1, 256:511], in0=t1[0:1, 256:511],
                        in1=psu[64:65, 257:512], op=ADD,
                    )

            def prev_out(pv, half):
                b, t1 = pv["b"], pv["t1"]
                lo = 256 * half
                nc.scalar.activation(
                    out=out_sb[0:1, b * D + lo:b * D + lo + 256],
                    in_=t1[0:1, lo:lo + 256],
                    func=Tanh, bias=zbias[0:1, :],
                )
                if half == 1:
                    nc.scalar.dma_start(
                        out=out[0:1, b * D:(b + 1) * D],
                        in_=out_sb[0:1, b * D:(b + 1) * D],
                    )

            prev_slots = [
                prev_tail45,
                lambda pv: prev_mix(pv, 0),
                lambda pv: (prev_mix(pv, 1), prev_out(pv, 0)),
                lambda pv: prev_out(pv, 1),
            ]

            pend = None
            for b in range(BPC):
                if b == 0:
                    encTt, encNt = encTt0, encNt0
                else:
                    encTt = encp.tile([128, 4, LP], bf16, tag="encTt",
                                      name="encTt")
                    nc.sync.dma_start(out=encTt[:, :, :], in_=encT[b])
                    encNt = encp.tile([128, NLB, D], bf16, tag="encNt",
                                      name="encNt")
                    nc.gpsimd.dma_start(out=encNt[:, :, :], in_=encN[b])
                r6c = r6cs[b % 2]
                stat = stats[b % 2]
                psu = ps_u.tile([SW, 512], f32, tag="psu", name="psu")
                rlowpw = workp.tile([128, NLB, 1], f32, tag="rlowpw",
                                    name="rlowpw")
                r6s = workp.tile([128, NLB, 1], f32, tag="r6s", name="r6s")
                # strict-lower colsum accumulators: A covers m in [192, 704)
                # (nested-suffix accumulation, one matmul per strip), B covers
                # m in [128, 192) (strip 0 only)
                slowA = ps_sl.tile([1, 512], f32, tag="slowA", name="slowA")
                slowB = ps_sl.tile([1, 64], f32, tag="slowB", name="slowB")

                def emit_ones(l, tsb_l):
                    # column sums of strip l feed the lower part of later rows
                    M = LBS[l]
                    # every matmul closes its own (sim-level) group so the
                    # per-block bounce reads may interleave with later strips'
                    # accumulation; start=False keeps accumulating on hardware
                    if l == 0:
                        nc.tensor.matmul(
                            slowB[0:1, 0:64], ones_sb[0:M, 0:1],
                            tsb_l[0:M, 128:192], start=True, stop=True,
                        )
                        nc.tensor.matmul(
                            slowA[0:1, 0:512], ones_sb[0:M, 0:1],
                            tsb_l[0:M, 192:704], start=True, stop=True,
                            skip_group_check=True,
                        )
                    else:  # m in [128(l+1), 704) -> A cols [128l-64:512)
                        nc.tensor.matmul(
                            slowA[0:1, 128 * l - 64:512], ones_sb[0:M, 0:1],
                            tsb_l[0:M, 128:704 - 128 * l],
                            start=False, stop=True,
                            skip_group_check=True,
                        )

                def block_rlow(j):
                    # bounce row0 block j <- scaled colsums + center-tap row,
                    # 32x32 block transpose, gather to 128-partition layout
                    c0 = 128 * j
                    if j == 1:
                        nc.vector.scalar_tensor_tensor(
                            out=bounce[0:1, 128:192], in0=slowB[0:1, :],
                            scalar=RSCALE, in1=w3ct_sb[0:1, 128:192],
                            op0=MULT, op1=ADD,
                        )
                        nc.vector.scalar_tensor_tensor(
                            out=bounce[0:1, 192:256], in0=slowA[0:1, 0:64],
                            scalar=RSCALE, in1=w3ct_sb[0:1, 192:256],
                            op0=MULT, op1=ADD,
                        )
                    else:
                        w = min(128, LP - c0)
                        nc.vector.scalar_tensor_tensor(
                            out=bounce[0:1, c0:c0 + w],
                            in0=slowA[0:1, c0 - 192:c0 - 192 + w],
                            scalar=RSCALE, in1=w3ct_sb[0:1, c0:c0 + w],
                            op0=MULT, op1=ADD,
                        )
                    nc.vector.transpose(out=outT[:, c0:c0 + 128],
                                        in_=bounce[:, c0:c0 + 128])
                    outT_v = outT.rearrange("p (c k) -> p c k", k=128)
                    for q in range(4):
                        nc.gpsimd.tensor_copy(
                            out=rlowpw[32 * q:32 * q + 32, j:j + 1, :],
                            in_=outT_v[0:32, j:j + 1, 32 * q:32 * q + 1],
                        )

                def block_stat(j):
                    # stationary col0 of block j: r_total/(2L) + 2*W3u[:,1]
                    nc.vector.tensor_tensor(
                        out=r6s[:, j:j + 1, :], in0=r6c[:, j:j + 1, 0:1],
                        in1=r6c[:, j:j + 1, 1:2], op=ADD,
                    )
                    in1 = (w3u_sb[:, 0:1, 1:2] if j == 0
                           else rlowpw[:, j:j + 1, :])
                    nc.vector.scalar_tensor_tensor(
                        out=stat[:, j:j + 1, 0:1], in0=r6s[:, j:j + 1, :],
                        scalar=RSCALE, in1=in1, op0=MULT, op1=ADD,
                    )

                def tail_mm(lb):
                    K = LBS[lb] + (1 if lb == NLB - 1 else 0)
                    nc.tensor.matmul(
                        psu[:, :], stat[0:K, lb, :], encNt[0:K, lb, :],
                        start=(lb == 0), stop=(lb == NLB - 1),
                    )

                tsb_prev = None
                for lb in range(NLB):
                    M = LBS[lb]
                    mstart = 128 * lb
                    extent = LP - mstart
                    # upper-triangle strip: rows of l-block lb, m >= mstart;
                    # split >512 extents evenly so no chunk is LDW-bound
                    tsb = workp.tile([128, LP], bf16, tag="tsb", bufs=3,
                                     name="tsb")
                    if extent > 512:
                        half = (extent // 2 + 31) & ~31
                        chunks = [(mstart, mstart + half), (mstart + half, LP)]
                    else:
                        chunks = [(mstart, LP)]
                    for ci, (c0, c1) in enumerate(chunks):
                        pssc = ps_s.tile([128, c1 - c0], f32, tag="pss",
                                         name="pssc")
                        for dc in range(4):
                            nc.tensor.matmul(
                                pssc[0:M, :],
                                encTt[:, dc, mstart:mstart + M],
                                encTt[:, dc, c0:c1],
                                start=(dc == 0),
                                stop=(dc == 3),
                            )
                        if len(chunks) == 2:
                            # tanh + upper-part row sum in one ScalarE pass
                            nc.scalar.activation(
                                out=tsb[0:M, c0 - mstart:c1 - mstart],
                                in_=pssc[0:M, :],
                                func=Tanh,
                                scale=1.0 / TEMP,
                                bias=zbias[0:M, :],
                                accum_out=r6c[0:M, lb, ci:ci + 1],
                            )
                        else:
                            # single-chunk strips: plain tanh, row sum on DVE
                            nc.scalar.activation(
                                out=tsb[0:M, c0 - mstart:c1 - mstart],
                                in_=pssc[0:M, :],
                                func=Tanh,
                                scale=1.0 / TEMP,
                                bias=zbias[0:M, :],
                            )
                            nc.vector.tensor_reduce(
                                out=r6c[0:M, lb, 0:1],
                                in_=tsb[0:M, 0:extent],
                                axis=mybir.AxisListType.X,
                                op=ADD,
                            )
                    if lb >= 1:
                        emit_ones(lb - 1, tsb_prev)
                        block_rlow(lb)
                        block_stat(lb - 1)
                    if lb >= 2:
                        tail_mm(lb - 2)
                    if pend is not None and lb < len(prev_slots):
                        prev_slots[lb](pend)
                    tsb_prev = tsb

                block_stat(NLB - 1)
                pend = {"psu": psu, "stat": stat, "encNt": encNt, "b": b}
            for slot in prev_slots:
                slot(pend)
    nc.finalize()
    return nc


def _get_program():
    global _PROG
    if _PROG is None:
        _PROG = _build_program()
    return _PROG


def _host_prep(inputs):
    bf16 = ml_dtypes.bfloat16
    enc = np.asarray(inputs["enc_output"], dtype=np.float32)
    user = np.asarray(inputs["user_embeddings"], dtype=np.float32)
    cw = np.asarray(inputs["conv_w"], dtype=np.float32)[0, 0]      # [3, 3]
    cb = float(np.asarray(inputs["conv_b"], dtype=np.float32)[0])
    w3 = np.asarray(inputs["conv3_w"], dtype=np.float32)[0, 0, :, 0]  # [700]
    c3b = float(np.asarray(inputs["conv3_b"], dtype=np.float32)[0])

    const = cb * float(w3.sum()) + c3b
    userp = (user + 2.0 * const).astype(np.float32)

    encP = np.zeros((B, LW, D), dtype=np.float32)
    encP[:, :L, :] = enc
    # row 704 carries the user embedding: the fused tail matmul picks it up
    # with stationary col0 weight 1.0 (via w3ct[704]), folding the user add
    # into the same PSUM accumulation
    encP[:, LP, :] = userp
    # encN[b, p, c, d] = enc[b, c*128+p, d]  (partition-contiguous layout)
    encN_h = np.ascontiguousarray(
        encP.reshape(B, NLB, 128, D).transpose(0, 2, 1, 3)).astype(bf16)
    # encT[b, p, c, m] = enc[b, m, c*128+p]
    encT_h = np.ascontiguousarray(
        encP[:, :LP].reshape(B, LP, 4, 128).transpose(0, 3, 2, 1)).astype(bf16)

    # W3u[l, j] = sum_i cw[i, j] * w3[l + 1 - i]; doubled (the 2*seq2 factor)
    W3u = np.zeros((LW, 3), dtype=np.float32)
    lidx = np.arange(L)
    for j in range(3):
        for i in range(3):
            src = lidx + 1 - i
            valid = (src >= 0) & (src < L)
            W3u[lidx[valid], j] += cw[i, j] * w3[src[valid]]
    W3u *= 2.0
    # w3u[p, c, j] = W3u[c*128+p, j]
    w3u_bf = np.ascontiguousarray(
        W3u.reshape(NLB, 128, 3).transpose(1, 0, 2)).astype(bf16)
    w3ct_h = np.ascontiguousarray(W3u[:, 1]).reshape(1, LW).astype(np.float32)
    w3ct_h[0, LP] = 1.0  # weight for the user-embedding row

    in_maps = []
    for c in range(NCORES):
        s = slice(c * BPC, (c + 1) * BPC)
        in_maps.append({
            "encN": encN_h[s],
            "encT": encT_h[s],
            "w3u": w3u_bf,
            "w3ct": w3ct_h,
        })
    return in_maps


def kernel(**inputs) -> np.ndarray:
    from concourse.bass_utils import run_bass_kernel_spmd

    in_maps = _host_prep(inputs)
    res = run_bass_kernel_spmd(_get_program(), in_maps, list(range(NCORES)))
    outs = [np.asarray(res.results[c]["out"], dtype=np.float32).reshape(BPC, D)
            for c in range(NCORES)]
    return np.concatenate(outs, axis=0)
